# revision 38
# baseline (speedup 1.0000x reference)
"""Causal Conv1d (K=4) + bias + silu for TRN2 via Winograd F(4,4), 8 cores.

Reference op: x (B=4, C_IN=2048, S=4096) fp32, weight (C_OUT=2048, C_IN, 4),
bias (C_OUT,);  out = silu(causal_conv1d(x, weight) + bias).

Sharding: data-parallel over sequence; core c computes out[:, :, c*512:(c+1)*512]
from x[:, :, c*512-3 : c*512+512] (zero-padded left halo), full weight/bias.

Algorithm: Winograd/Toom-Cook F(4,4) over the sequence dim with 7 finite
points {0, +-1, +-2, +-1/2}. Each tile of m=4 outputs needs U=7 transformed
products instead of 16 direct MACs: PE work drops to 7/16 of direct conv
(1792 accumulating 128x128x512 fp16 matmuls per core instead of 4096).

  y = A^T [ (G w) . (B^T d) ]   per tile of 4 outputs, 7-point window d
  B^T rows = Lagrange numerator polys (dense, evaluated on DVE with
             even/odd CSE: 21 fp16 ops per k-tile)
  A^T      = Vandermonde powers (incremental S/D combine on DVE, fp16)
  G w      = host-precomputed fp16 weights (streamed, 7/4 x direct size @fp16)

Pipeline: 3 stages over point-pairs {+-1}, {+-2}, {+-1/2, 0}. Per stage the
DVE transforms all 16 k-tiles (BX, fp16), then PE accumulates per (mi, u)
PSUM groups which DVE combines (Vandermonde) into per-mi fp16 y tiles.
Stages of mi-loop matmuls overlap the next stage's input transform. Act does
the final strided-gather silu+bias into fp16 out tiles (host upcasts).

v2 schedule (shipped; cost-model-sim overhead 25.4us vs spread4's 48.8us):
 - weight/bias stream on the Pool SWDGE queue, planes + half the outputs on
   SP HWDGE, other outputs on Pool: the spread4 prologue was 41.7us of SP
   FIFO (16 plane DMAs queued before the first weight tile).
 - wpool ring 8 deep + 4 migroups prefetched: 8 PSUM chains ride the
   stage-0 plane-arrival window (PE cap there = 8 banks x 512 cols
   = 1.7us/plane vs 2.37us plane DMA; that residual drift is the
   single-DMA-queue floor).
 - single Act eviction per migroup (pa only); DVE reads pb/pc straight
   from PSUM (walrus allows one PSUM operand per DVE op). Stage-0 y2/y3
   copies are deferred into stage 1 (reads y0/y1 pre-update).
 - stage-0 odd-plane chain on Pool for ki>=5 via ISA-valid TensorScalar +
   TensorTensor (fused TensorScalarPtr fails the Pool ISA check), keeping
   DVE's per-ki cost under the plane cadence.
 - first post-window weight tile loads in two ki-half chunks (its full
   1.6us DMA sat on the PE critical path).
 - stage-2 combine orders y0 last (only y0 waits on the u6 chain); silu
   splits into an early j=1..3 3D-AP op and a late j=0 op per batch, so
   only j=0 trails the final matmul.

Numerics: x/BX/GW/y in fp16, PSUM fp32, out fp16->fp32 on host. Measured
error ~4.5e-3 of max|out| (gate 2e-2); fp16 matmul = f32r PE rate.
"""

import numpy as np

import concourse.bacc as bacc
import concourse.bass as bass
import concourse.mybir as mybir
import concourse.tile as tile
from concourse.alu_op_type import AluOpType
from concourse.bass_utils import run_bass_kernel_spmd

P = 128

B = 4
C_IN = 2048
C_OUT = 2048
KTAPS = 4
S = 4096
N_CORES = 8
S_CHUNK = S // N_CORES          # 512
HALO = KTAPS - 1                # 3

M_TILE = 4                       # F(4,4): outputs per winograd tile
U = 7                            # transform size
NT = S_CHUNK // M_TILE           # 128 winograd tiles per batch
FD = B * NT                      # 512 matmul free dim (4 batches side by side)
N_KI = C_IN // P                 # 16
N_MI = C_OUT // P                # 16

# Point order in weight/BX memory: [+1, -1, +1/2, -1/2, +2, -2, 0]
# (stage grouping: stage0={+1,-1}, stage1={+1/2,-1/2}, stage2={+2,-2,0})
POINTS_MEM = [1.0, -1.0, 0.5, -0.5, 2.0, -2.0, 0.0]
STAGES = [(0, 2), (2, 4), (4, 7)]   # u-ranges per stage


def winograd_G():
    """G (U x r) for points POINTS_MEM (last = 0), r=4, exact rationals."""
    from fractions import Fraction as F
    pts = [F(1), F(-1), F(1, 2), F(-1, 2), F(2), F(-2), F(0)]
    G = []
    for u, p in enumerate(pts):
        den = F(1)
        for k, q in enumerate(pts):
            if k != u:
                den *= (p - q)
        G.append([p ** t / den for t in range(KTAPS)])
    return np.array([[float(c) for c in row] for row in G])


def build_winograd_nc(reps=1, schedule="spread4"):
    f16 = mybir.dt.float16
    f32 = mybir.dt.float32
    MUL, ADD, SUB = AluOpType.mult, AluOpType.add, AluOpType.subtract
    silu_fn = mybir.ActivationFunctionType.Silu

    nc = bacc.Bacc("TRN2", target_bir_lowering=False, debug=False)
    # v2: fp16 device output (host upcasts); halves the out-DMA bytes and
    # lets out tiles go out over either DMA queue without a casting SWDGE
    o_dram_dt = f16 if schedule.startswith("v2") else f32

    # x planes: [ki, p, v(7), bi(4), t(128)] fp16; plane v holds x_pad[4t+v]
    xp_d = nc.dram_tensor(
        "x", [N_KI, P, U, B, NT], f16, kind="ExternalInput"
    ).ap()
    # winograd weights: [mi, p(ci), u, ki, f(co)] fp16
    w_d = nc.dram_tensor(
        "w", [N_MI, P, U, N_KI, P], f16, kind="ExternalInput"
    ).ap()
    bias_d = nc.dram_tensor("bias", [P, N_MI], f32, kind="ExternalInput").ap()
    out_d = nc.dram_tensor(
        "out", [B, N_MI * P, S_CHUNK], o_dram_dt, kind="ExternalOutput"
    ).ap()

    ps_banks = [
        nc.alloc_psum_tensor(f"psb{k}", [P, FD], f32).ap() for k in range(8)
    ]
    bank_ctr = [0]

    def next_bank():
        b = ps_banks[bank_ctr[0] % 8]
        bank_ctr[0] += 1
        return b

    def stt(out, in0, scalar, in1):
        nc.vector.scalar_tensor_tensor(out, in0, scalar, in1, MUL, ADD)

    def tt(out, in0, in1, op):
        nc.vector.tensor_tensor(out, in0, in1, op)

    def g_stt(out, in0, scalar, in1):
        nc.gpsimd.scalar_tensor_tensor(out, in0, scalar, in1, MUL, ADD)

    def g_tt(out, in0, in1, op):
        nc.gpsimd.tensor_tensor(out, in0, in1, op)

    # v2 schedules: weight stream rides the Pool SWDGE queue so it never
    # queues behind the x-plane loads on SP's HWDGE FIFO (the spread4
    # prologue was 41.7us of SP FIFO: 16 plane DMAs before the first w).
    w_dma = nc.gpsimd.dma_start if schedule.startswith("v2") else nc.sync.dma_start

    with tile.TileContext(nc) as tc:
        if schedule.startswith("v2"):
            n_pl, n_w, n_o = 3, 8, 4
        else:
            n_pl = 4 if schedule in ("spread4", "spread5") else 3
            n_w = 5 if schedule in ("spread2", "spread3") else (4 if schedule == "spread5" else 3)
            n_o = 3 if schedule == "spread5" else 4
        with (
            tc.tile_pool(name="plpool", bufs=n_pl) as plpool,
            tc.tile_pool(name="bxpool", bufs=2) as bxpool,
            tc.tile_pool(name="bzpool", bufs=1) as bzpool,
            tc.tile_pool(name="ypool", bufs=1) as ypool,
            tc.tile_pool(name="wpool", bufs=n_w) as wpool,
            tc.tile_pool(name="opool", bufs=n_o) as opool,
            tc.tile_pool(name="scpool", bufs=2) as scpool,
            tc.tile_pool(name="sdpool", bufs=1) as sdpool,
            tc.tile_pool(name="bpool", bufs=1) as bpool,
        ):
            # s2/d2 y-combine transients and transform osc scratch:
            # single-buffered pool (consumers immediately follow producers
            # in DVE program order; saves 3KB/partition)
            sd_pool = sdpool if schedule.startswith("v2") else scpool
            osc_pool = sdpool if schedule.startswith("v2") else scpool
            bias_t = bpool.tile([P, N_MI], f32, tag="bias")
            if not schedule.startswith("v2"):
                # v2 emits this after the stage-0 weight prefetch (bias is
                # first needed by stage-2 silu, much later)
                w_dma(out=bias_t, in_=bias_d)

            for rep in range(reps):
                y_t = {}
                for mi in range(N_MI):
                    y_t[mi] = ypool.tile([P, M_TILE * FD], f16, tag=f"y{mi}", name=f"y{mi}")

                # bx[(stage, ki, uu)] -> AP of transformed input
                bx = {}

                def emit_transform(sts, ki):
                    if (schedule in ("spread4", "spread5")
                            or schedule.startswith("v2")) and sts == [0]:
                        # stage 0 uses only planes d1..d6: slim DMA off
                        # the prologue's critical chain
                        pl = plpool.tile([P, 6 * FD], f16, tag="pl", name="pl")
                        nc.sync.dma_start(out=pl, in_=xp_d[ki, :, 1:7])
                        d = [None] + [pl[:, v * FD:(v + 1) * FD]
                                      for v in range(6)]
                    else:
                        pl = plpool.tile([P, U * FD], f16, tag="pl", name="pl")
                        nc.sync.dma_start(out=pl, in_=xp_d[ki])
                        d = [pl[:, v * FD:(v + 1) * FD] for v in range(U)]
                    for st in sts:
                        bx_t = bxpool.tile(
                            [P, 2 * FD], f16, tag=f"bx{ki}", name=f"bx{ki}"
                        )
                        ep = bx_t[:, 0:FD]
                        em = bx_t[:, FD:2 * FD]
                        o = osc_pool.tile([P, FD], f16, tag="osc", name="osc")
                        if st == 0:         # points +-1
                            stt(ep, d[4], -4.25, d[2])
                            tt(ep, ep, d[6], ADD)
                            if schedule == "spread3":
                                # O-chain on Pool: halves the stage-0
                                # transform wave that gates PE start
                                g_stt(o, d[3], -4.25, d[1])
                                g_tt(o, o, d[5], ADD)
                            elif schedule.startswith("v2") and ki >= 5:
                                # O-chain on Pool via ISA-valid ops only
                                # (TensorScalar + TensorTensor; the fused
                                # TensorScalarPtr fails the Pool ISA
                                # check in walrus codegen): drops DVE's
                                # per-ki cost below the plane-DMA cadence
                                # so the plpool ring never stalls SP.
                                # ki<5 stays on DVE: Pool is busy with
                                # the w-prefetch DMAs until ~13us.
                                nc.gpsimd.tensor_scalar_mul(o, d[3], -4.25)
                                g_tt(o, o, d[1], ADD)
                                g_tt(o, o, d[5], ADD)
                            else:
                                stt(o, d[3], -4.25, d[1])
                                tt(o, o, d[5], ADD)
                            tt(em, ep, o, SUB)
                            tt(ep, ep, o, ADD)
                        elif st == 1:       # points +-1/2
                            stt(ep, d[2], 4.0, d[6])
                            stt(ep, d[4], -5.0, ep)
                            stt(o, d[3], -5.0, d[5])
                            stt(o, d[1], 4.0, o)
                            stt(em, o, -0.5, ep)
                            stt(ep, o, 0.5, ep)
                        else:               # points +-2 and 0
                            stt(ep, d[2], 0.25, d[6])
                            stt(ep, d[4], -1.25, ep)
                            stt(o, d[3], -5.0, d[1])
                            stt(o, d[5], 4.0, o)
                            stt(em, o, -0.5, ep)
                            stt(ep, o, 0.5, ep)
                            bz = bzpool.tile(
                                [P, FD], f16, tag=f"bz{ki}", name=f"bz{ki}"
                            )
                            tt(bz, d[6], d[0], SUB)
                            o2 = osc_pool.tile([P, FD], f16, tag="osc", name="osc")
                            tt(o2, d[2], d[4], SUB)
                            stt(bz, o2, 5.25, bz)
                            bx[st, ki, 2] = bz
                        bx[st, ki, 0] = bx_t[:, 0:FD]
                        bx[st, ki, 1] = bx_t[:, FD:2 * FD]

                pre_w = {}

                def prefetch_w(stage, mi):
                    u_lo, u_hi = STAGES[stage]
                    for uu in range(u_hi - u_lo):
                        w_t = wpool.tile([P, N_KI * P], f16, tag="w", name="w")
                        w_dma(
                            out=w_t, in_=w_d[mi, :, u_lo + uu, :, :]
                        )
                        pre_w[stage, mi, uu] = w_t

                def emit_migroup(stage, mi):
                    u_lo, u_hi = STAGES[stage]
                    n_u = u_hi - u_lo
                    if True:
                        psums = []
                        for uu in range(n_u):
                            w_t = pre_w.pop((stage, mi, uu), None)
                            if w_t is None:
                                w_t = wpool.tile([P, N_KI * P], f16, tag="w", name="w")
                                if schedule.startswith("v2") and stage == 0:
                                    # two ki-half chunks: the chain's ki=0
                                    # matmul only waits on the first half
                                    # (the first post-window w load sits on
                                    # the PE critical path)
                                    h = N_KI // 2 * P
                                    w_dma(
                                        out=w_t[:, 0:h],
                                        in_=w_d[mi, :, u_lo + uu, 0:N_KI // 2, :],
                                    )
                                    w_dma(
                                        out=w_t[:, h:2 * h],
                                        in_=w_d[mi, :, u_lo + uu, N_KI // 2:, :],
                                    )
                                else:
                                    w_dma(
                                        out=w_t, in_=w_d[mi, :, u_lo + uu, :, :]
                                    )
                            ps = next_bank()
                            psums.append(ps)
                            for ki in range(N_KI):
                                lhsT = w_t[:, ki * P:(ki + 1) * P]
                                nc.tensor.matmul(
                                    ps, lhsT, bx[stage, ki, uu],
                                    start=(ki == 0), stop=(ki == N_KI - 1),
                                )
                        y = y_t[mi]
                        yj = [y[:, j * FD:(j + 1) * FD] for j in range(M_TILE)]
                        if schedule.startswith("v2"):
                            # one Act eviction per migroup: pa -> SBUF; the
                            # DVE combine reads pb/pc straight from PSUM
                            # (walrus allows ONE PSUM operand per DVE op).
                            # Stage 0 skips the y2/y3 copies: stage 1 reads
                            # y0/y1 (pre-update) to seed them instead.
                            m_t = scpool.tile([P, FD], f16, tag="m0")
                            nc.scalar.copy(m_t, psums[0])
                            if stage == 0 and mi < 4:
                                # window-wave migroups: evict pb via Act too
                                # (Act is idle here; DVE reading PSUM would
                                # delay the first post-window bank release)
                                pb = sd_pool.tile([P, FD], f16, tag="mb")
                                nc.scalar.copy(pb, psums[1])
                            else:
                                pb = psums[1]
                            if stage == 0:
                                tt(yj[0], m_t, pb, ADD)
                                tt(yj[1], m_t, pb, SUB)
                            elif stage == 1:
                                s2 = sd_pool.tile([P, FD], f16, tag="s2")
                                d2 = sd_pool.tile([P, FD], f16, tag="d2")
                                tt(s2, m_t, pb, ADD)
                                tt(d2, m_t, pb, SUB)
                                stt(yj[2], s2, 0.25, yj[0])
                                tt(yj[0], s2, yj[0], ADD)
                                stt(yj[3], d2, 0.125, yj[1])
                                stt(yj[1], d2, 0.5, yj[1])
                            else:
                                sh = sd_pool.tile([P, FD], f16, tag="s2")
                                dh = sd_pool.tile([P, FD], f16, tag="d2")
                                tt(sh, m_t, pb, ADD)
                                tt(dh, m_t, pb, SUB)
                                stt(yj[2], sh, 4.0, yj[2])
                                stt(yj[1], dh, 2.0, yj[1])
                                stt(yj[3], dh, 8.0, yj[3])
                                # y0 last: it alone waits on the u6 chain
                                # (psums[2]); y1..y3 close before the last
                                # matmul so their silus overlap it
                                tt(yj[0], sh, yj[0], ADD)
                                tt(yj[0], yj[0], psums[2], ADD)
                        else:
                            # Act evicts each closed PSUM group to fp16 SBUF
                            # (walrus: only one PSUM operand per DVE op)
                            mts = []
                            for uu in range(n_u):
                                m_t = scpool.tile([P, FD], f16, tag=f"m{uu}")
                                nc.scalar.copy(m_t, psums[uu])
                                mts.append(m_t)
                            pa, pb = mts[0], mts[1]
                            if stage == 0:
                                tt(yj[0], pa, pb, ADD)
                                tt(yj[1], pa, pb, SUB)
                                nc.vector.tensor_copy(yj[2], yj[0])
                                nc.vector.tensor_copy(yj[3], yj[1])
                            elif stage == 1:
                                s2 = sd_pool.tile([P, FD], f16, tag="s2")
                                d2 = sd_pool.tile([P, FD], f16, tag="d2")
                                tt(s2, pa, pb, ADD)
                                tt(d2, pa, pb, SUB)
                                tt(yj[0], s2, yj[0], ADD)
                                stt(yj[2], s2, 0.25, yj[2])
                                stt(yj[1], d2, 0.5, yj[1])
                                stt(yj[3], d2, 0.125, yj[3])
                            else:
                                sh = sd_pool.tile([P, FD], f16, tag="s2")
                                dh = sd_pool.tile([P, FD], f16, tag="d2")
                                tt(sh, pa, pb, ADD)
                                tt(dh, pa, pb, SUB)
                                tt(yj[0], sh, yj[0], ADD)
                                tt(yj[0], mts[2], yj[0], ADD)
                                stt(yj[2], sh, 4.0, yj[2])
                                stt(yj[1], dh, 2.0, yj[1])
                                stt(yj[3], dh, 8.0, yj[3])
                        if stage == 2:
                            # finished: silu + bias, de-interleave, store
                            if schedule.startswith("v2"):
                                # two Act ops per bi: j=1..3 (ready before
                                # the u6 chain closes) then j=0 (waits on
                                # y0's psum add); fp16 out tiles; out DMAs
                                # alternate SP/Pool so the tail pipelines
                                for bi in range(B):
                                    o_t = opool.tile(
                                        [P, S_CHUNK], f16, tag="o"
                                    )
                                    # j=1..3 in one 3D-AP op: out dims
                                    # [p][j:3,s=1][t:NT,s=4], in dims
                                    # [p][j:3,s=FD][t:NT,s=1]
                                    o123 = bass.AP(
                                        o_t.tensor, o_t.offset + 1,
                                        [list(o_t.ap[0]), [1, 3],
                                         [M_TILE, NT]],
                                    )
                                    y123 = bass.AP(
                                        y.tensor,
                                        y.offset + FD + bi * NT,
                                        [list(y.ap[0]), [FD, 3], [1, NT]],
                                    )
                                    nc.scalar.activation(
                                        o123, y123,
                                        silu_fn,
                                        bias=bias_t[:, mi:mi + 1],
                                    )
                                    nc.scalar.activation(
                                        o_t[:, 0:S_CHUNK:M_TILE],
                                        y[:, bi * NT:(bi + 1) * NT],
                                        silu_fn,
                                        bias=bias_t[:, mi:mi + 1],
                                    )
                                    odma = (nc.sync.dma_start if bi % 2 == 0
                                            else nc.gpsimd.dma_start)
                                    odma(
                                        out=out_d[bi,
                                                  mi * P:(mi + 1) * P, :],
                                        in_=o_t,
                                    )
                            else:
                                for bi in range(B):
                                    o_t = opool.tile(
                                        [P, S_CHUNK], f32, tag="o"
                                    )
                                    for j in range(M_TILE):
                                        nc.scalar.activation(
                                            o_t[:, j:S_CHUNK:M_TILE],
                                            y[:, j * FD + bi * NT:
                                               j * FD + (bi + 1) * NT],
                                            silu_fn,
                                            bias=bias_t[:, mi:mi + 1],
                                        )
                                    nc.sync.dma_start(
                                        out=out_d[bi,
                                                  mi * P:(mi + 1) * P, :],
                                        in_=o_t,
                                    )

                if schedule == "fused":
                    # one plane load feeds stage0+stage1 transforms up
                    # front; stage2 transforms interleave into stage1's
                    # matmul loop (PE never waits at stage boundaries,
                    # plane DMA read twice per pass).
                    for ki in range(N_KI):
                        emit_transform([0, 1], ki)
                    for mi in range(N_MI):
                        emit_migroup(0, mi)
                    for mi in range(N_MI):
                        emit_migroup(1, mi)
                        emit_transform([2], mi)
                    for mi in range(N_MI):
                        emit_migroup(2, mi)
                else:
                    # spread: only stage0 transforms up front (shortest
                    # PE fill); stage s+1 transforms interleave into
                    # stage s's matmul loop (plane DMA read 3x per pass).
                    if schedule in ("spread2", "spread3", "spread5"):
                        # first two mi's weights ahead of the plane DMAs
                        # on SP's FIFO so PE can start with the first BX
                        prefetch_w(0, 0)
                        prefetch_w(0, 1)
                    elif schedule.startswith("v2"):
                        # Pool queue runs parallel to SP: prefetch the
                        # first four migroups' weights (fills the 8-buf
                        # ring: 8 chains ride the plane-arrival window)
                        for pmi in range(4):
                            prefetch_w(0, pmi)
                        w_dma(out=bias_t, in_=bias_d)
                        # warm up Act off the critical path: its first op
                        # costs ~2us (pipe ramp) and otherwise lands on the
                        # first post-window PSUM eviction
                        warm_t = bpool.tile([P, N_MI], f32, tag="warm")
                        nc.scalar.copy(warm_t, bias_t)
                    for ki in range(N_KI):
                        emit_transform([0], ki)
                    for mi in range(N_MI):
                        emit_migroup(0, mi)
                        emit_transform([1], mi)
                    for mi in range(N_MI):
                        emit_migroup(1, mi)
                        emit_transform([2], mi)
                    for mi in range(N_MI):
                        emit_migroup(2, mi)
    nc.compile()
    return nc


def prep_inputs(x, weight, bias):
    """Full fp32 inputs -> per-core in_maps with winograd host transforms."""
    x = np.asarray(x, dtype=np.float32)
    weight = np.asarray(weight, dtype=np.float32)
    bias = np.asarray(bias, dtype=np.float32)

    G = winograd_G()
    # GW[u, co, ci] fp16, laid out [mi, p(ci), u, ki, f(co)]
    GW = np.einsum('ut,oit->uoi', G, weight).astype(np.float16)
    GW = GW.reshape(U, N_MI, P, N_KI, P)          # (u, mi, f, ki, p)
    w_host = np.ascontiguousarray(GW.transpose(1, 4, 0, 3, 2))

    bias2 = np.ascontiguousarray(bias.reshape(N_MI, P).T)  # (P, n_mi)

    xp = np.pad(x, ((0, 0), (0, 0), (HALO, 0))).astype(np.float16)  # (B,CI,S+3)
    in_maps = []
    for c in range(N_CORES):
        xc = xp[:, :, c * S_CHUNK: c * S_CHUNK + S_CHUNK + HALO]  # (B,CI,515)
        # planes[v][t] = xc[..., 4t+v], t<128 -> layout [ki,p,v,bi,t]
        pl = np.empty((B, C_IN, U, NT), dtype=np.float16)
        for v in range(U):
            pl[:, :, v, :] = xc[:, :, v: v + 4 * NT: 4][:, :, :NT]
        pl = pl.reshape(B, N_KI, P, U, NT)
        pl = np.ascontiguousarray(pl.transpose(1, 2, 3, 0, 4))  # ki,p,v,bi,t
        in_maps.append({"x": pl, "w": w_host, "bias": bias2})
    return in_maps


def build_for_bench(x, weight, bias, reps=1, schedule="v2"):
    nc = build_winograd_nc(reps=reps, schedule=schedule)
    in_maps = prep_inputs(x, weight, bias)
    return nc, in_maps


def kernel(x, weight, bias):
    nc, in_maps = build_for_bench(x, weight, bias, reps=1)
    global LAST_RESULT
    res = run_bass_kernel_spmd(
        nc, in_maps, core_ids=list(range(N_CORES)), trace=PROFILE
    )
    LAST_RESULT = res
    out = np.concatenate([r["out"] for r in res.results], axis=2)
    return np.ascontiguousarray(out.astype(np.float32, copy=False))


PROFILE = False
LAST_RESULT = None



# revision 39
# speedup vs baseline: 1.0019x; 1.0019x over previous
"""Causal Conv1d (K=4) + bias + silu for TRN2 via Winograd F(4,4), 8 cores.

Reference op: x (B=4, C_IN=2048, S=4096) fp32, weight (C_OUT=2048, C_IN, 4),
bias (C_OUT,);  out = silu(causal_conv1d(x, weight) + bias).

Sharding: data-parallel over sequence; core c computes out[:, :, c*512:(c+1)*512]
from x[:, :, c*512-3 : c*512+512] (zero-padded left halo), full weight/bias.

Algorithm: Winograd/Toom-Cook F(4,4) over the sequence dim with 7 finite
points {0, +-1, +-2, +-1/2}. Each tile of m=4 outputs needs U=7 transformed
products instead of 16 direct MACs: PE work drops to 7/16 of direct conv
(1792 accumulating 128x128x512 fp16 matmuls per core instead of 4096).

  y = A^T [ (G w) . (B^T d) ]   per tile of 4 outputs, 7-point window d
  B^T rows = Lagrange numerator polys (dense, evaluated on DVE with
             even/odd CSE: 21 fp16 ops per k-tile)
  A^T      = Vandermonde powers (incremental S/D combine on DVE, fp16)
  G w      = host-precomputed fp16 weights (streamed, 7/4 x direct size @fp16)

Pipeline: 3 stages over point-pairs {+-1}, {+-2}, {+-1/2, 0}. Per stage the
DVE transforms all 16 k-tiles (BX, fp16), then PE accumulates per (mi, u)
PSUM groups which DVE combines (Vandermonde) into per-mi fp16 y tiles.
Stages of mi-loop matmuls overlap the next stage's input transform. Act does
the final strided-gather silu+bias into fp16 out tiles (host upcasts).

v2 schedule (shipped; cost-model-sim overhead 25.4us vs spread4's 48.8us):
 - weight/bias stream on the Pool SWDGE queue, planes + half the outputs on
   SP HWDGE, other outputs on Pool: the spread4 prologue was 41.7us of SP
   FIFO (16 plane DMAs queued before the first weight tile).
 - wpool ring 8 deep + 4 migroups prefetched: 8 PSUM chains ride the
   stage-0 plane-arrival window (PE cap there = 8 banks x 512 cols
   = 1.7us/plane vs 2.37us plane DMA; that residual drift is the
   single-DMA-queue floor).
 - single Act eviction per migroup (pa only); DVE reads pb/pc straight
   from PSUM (walrus allows one PSUM operand per DVE op). Stage-0 y2/y3
   copies are deferred into stage 1 (reads y0/y1 pre-update).
 - stage-0 odd-plane chain on Pool for ki>=5 via ISA-valid TensorScalar +
   TensorTensor (fused TensorScalarPtr fails the Pool ISA check), keeping
   DVE's per-ki cost under the plane cadence.
 - first post-window weight tile loads in two ki-half chunks (its full
   1.6us DMA sat on the PE critical path).
 - stage-2 combine orders y0 last (only y0 waits on the u6 chain); silu
   splits into an early j=1..3 3D-AP op and a late j=0 op per batch, so
   only j=0 trails the final matmul.

Numerics: x/BX/GW/y in fp16, PSUM fp32, out fp16->fp32 on host. Measured
error ~4.5e-3 of max|out| (gate 2e-2); fp16 matmul = f32r PE rate.
"""

import numpy as np

import concourse.bacc as bacc
import concourse.bass as bass
import concourse.mybir as mybir
import concourse.tile as tile
from concourse.alu_op_type import AluOpType
from concourse.bass_utils import run_bass_kernel_spmd

P = 128

B = 4
C_IN = 2048
C_OUT = 2048
KTAPS = 4
S = 4096
N_CORES = 8
S_CHUNK = S // N_CORES          # 512
HALO = KTAPS - 1                # 3

M_TILE = 4                       # F(4,4): outputs per winograd tile
U = 7                            # transform size
NT = S_CHUNK // M_TILE           # 128 winograd tiles per batch
FD = B * NT                      # 512 matmul free dim (4 batches side by side)
N_KI = C_IN // P                 # 16
N_MI = C_OUT // P                # 16

# Point order in weight/BX memory: [+1, -1, +1/2, -1/2, +2, -2, 0]
# (stage grouping: stage0={+1,-1}, stage1={+1/2,-1/2}, stage2={+2,-2,0})
POINTS_MEM = [1.0, -1.0, 0.5, -0.5, 2.0, -2.0, 0.0]
STAGES = [(0, 2), (2, 4), (4, 7)]   # u-ranges per stage


def winograd_G():
    """G (U x r) for points POINTS_MEM (last = 0), r=4, exact rationals."""
    from fractions import Fraction as F
    pts = [F(1), F(-1), F(1, 2), F(-1, 2), F(2), F(-2), F(0)]
    G = []
    for u, p in enumerate(pts):
        den = F(1)
        for k, q in enumerate(pts):
            if k != u:
                den *= (p - q)
        G.append([p ** t / den for t in range(KTAPS)])
    return np.array([[float(c) for c in row] for row in G])


def build_winograd_nc(reps=1, schedule="spread4"):
    f16 = mybir.dt.float16
    f32 = mybir.dt.float32
    MUL, ADD, SUB = AluOpType.mult, AluOpType.add, AluOpType.subtract
    silu_fn = mybir.ActivationFunctionType.Silu

    nc = bacc.Bacc("TRN2", target_bir_lowering=False, debug=False)
    # v2: fp16 device output (host upcasts); halves the out-DMA bytes and
    # lets out tiles go out over either DMA queue without a casting SWDGE
    o_dram_dt = f16 if schedule.startswith("v2") else f32

    # x planes: [ki, p, v(7), bi(4), t(128)] fp16; plane v holds x_pad[4t+v]
    xp_d = nc.dram_tensor(
        "x", [N_KI, P, U, B, NT], f16, kind="ExternalInput"
    ).ap()
    # winograd weights: [mi, p(ci), u, ki, f(co)] fp16
    w_d = nc.dram_tensor(
        "w", [N_MI, P, U, N_KI, P], f16, kind="ExternalInput"
    ).ap()
    bias_d = nc.dram_tensor("bias", [P, N_MI], f32, kind="ExternalInput").ap()
    out_d = nc.dram_tensor(
        "out", [B, N_MI * P, S_CHUNK], o_dram_dt, kind="ExternalOutput"
    ).ap()

    ps_banks = [
        nc.alloc_psum_tensor(f"psb{k}", [P, FD], f32).ap() for k in range(8)
    ]
    bank_ctr = [0]

    def next_bank():
        b = ps_banks[bank_ctr[0] % 8]
        bank_ctr[0] += 1
        return b

    def stt(out, in0, scalar, in1):
        nc.vector.scalar_tensor_tensor(out, in0, scalar, in1, MUL, ADD)

    def tt(out, in0, in1, op):
        nc.vector.tensor_tensor(out, in0, in1, op)

    def g_stt(out, in0, scalar, in1):
        nc.gpsimd.scalar_tensor_tensor(out, in0, scalar, in1, MUL, ADD)

    def g_tt(out, in0, in1, op):
        nc.gpsimd.tensor_tensor(out, in0, in1, op)

    # v2 schedules: weight stream rides the Pool SWDGE queue so it never
    # queues behind the x-plane loads on SP's HWDGE FIFO (the spread4
    # prologue was 41.7us of SP FIFO: 16 plane DMAs before the first w).
    w_dma = nc.gpsimd.dma_start if schedule.startswith("v2") else nc.sync.dma_start

    with tile.TileContext(nc) as tc:
        if schedule.startswith("v2"):
            n_pl, n_w, n_o = 3, 8, 4
        else:
            n_pl = 4 if schedule in ("spread4", "spread5") else 3
            n_w = 5 if schedule in ("spread2", "spread3") else (4 if schedule == "spread5" else 3)
            n_o = 3 if schedule == "spread5" else 4
        with (
            tc.tile_pool(name="plpool", bufs=n_pl) as plpool,
            tc.tile_pool(name="bxpool", bufs=2) as bxpool,
            tc.tile_pool(name="bzpool", bufs=1) as bzpool,
            tc.tile_pool(name="ypool", bufs=1) as ypool,
            tc.tile_pool(name="wpool", bufs=n_w) as wpool,
            tc.tile_pool(name="opool", bufs=n_o) as opool,
            tc.tile_pool(name="scpool", bufs=2) as scpool,
            tc.tile_pool(name="sdpool", bufs=1) as sdpool,
            tc.tile_pool(name="bpool", bufs=1) as bpool,
        ):
            # s2/d2 y-combine transients and transform osc scratch:
            # single-buffered pool (consumers immediately follow producers
            # in DVE program order; saves 3KB/partition)
            sd_pool = sdpool if schedule.startswith("v2") else scpool
            osc_pool = sdpool if schedule.startswith("v2") else scpool
            bias_t = bpool.tile([P, N_MI], f32, tag="bias")
            if not schedule.startswith("v2"):
                # v2 emits this after the stage-0 weight prefetch (bias is
                # first needed by stage-2 silu, much later)
                w_dma(out=bias_t, in_=bias_d)

            for rep in range(reps):
                y_t = {}
                for mi in range(N_MI):
                    y_t[mi] = ypool.tile([P, M_TILE * FD], f16, tag=f"y{mi}", name=f"y{mi}")

                # bx[(stage, ki, uu)] -> AP of transformed input
                bx = {}

                def emit_transform(sts, ki):
                    if (schedule in ("spread4", "spread5")
                            or schedule.startswith("v2")) and sts == [0]:
                        # stage 0 uses only planes d1..d6: slim DMA off
                        # the prologue's critical chain
                        pl = plpool.tile([P, 6 * FD], f16, tag="pl", name="pl")
                        if schedule.startswith("v2") and ki == 0:
                            # first plane only: two contiguous halves on
                            # the two parallel HWDGE queues (Act is idle
                            # until ~15us). Plane0 lands ~0.6us sooner and
                            # the ki1..15 stream on SP starts that much
                            # earlier, shifting the whole DMA-bound
                            # window left.
                            nc.sync.dma_start(
                                out=pl[:, 0:3 * FD], in_=xp_d[ki, :, 1:4]
                            )
                            nc.scalar.dma_start(
                                out=pl[:, 3 * FD:6 * FD],
                                in_=xp_d[ki, :, 4:7],
                            )
                        else:
                            nc.sync.dma_start(out=pl, in_=xp_d[ki, :, 1:7])
                        d = [None] + [pl[:, v * FD:(v + 1) * FD]
                                      for v in range(6)]
                    else:
                        pl = plpool.tile([P, U * FD], f16, tag="pl", name="pl")
                        nc.sync.dma_start(out=pl, in_=xp_d[ki])
                        d = [pl[:, v * FD:(v + 1) * FD] for v in range(U)]
                    for st in sts:
                        bx_t = bxpool.tile(
                            [P, 2 * FD], f16, tag=f"bx{ki}", name=f"bx{ki}"
                        )
                        ep = bx_t[:, 0:FD]
                        em = bx_t[:, FD:2 * FD]
                        o = osc_pool.tile([P, FD], f16, tag="osc", name="osc")
                        if st == 0:         # points +-1
                            stt(ep, d[4], -4.25, d[2])
                            tt(ep, ep, d[6], ADD)
                            if schedule == "spread3":
                                # O-chain on Pool: halves the stage-0
                                # transform wave that gates PE start
                                g_stt(o, d[3], -4.25, d[1])
                                g_tt(o, o, d[5], ADD)
                            elif schedule.startswith("v2") and ki >= 5:
                                # O-chain on Pool via ISA-valid ops only
                                # (TensorScalar + TensorTensor; the fused
                                # TensorScalarPtr fails the Pool ISA
                                # check in walrus codegen): drops DVE's
                                # per-ki cost below the plane-DMA cadence
                                # so the plpool ring never stalls SP.
                                # ki<5 stays on DVE: Pool is busy with
                                # the w-prefetch DMAs until ~13us.
                                nc.gpsimd.tensor_scalar_mul(o, d[3], -4.25)
                                g_tt(o, o, d[1], ADD)
                                g_tt(o, o, d[5], ADD)
                            else:
                                stt(o, d[3], -4.25, d[1])
                                tt(o, o, d[5], ADD)
                            tt(em, ep, o, SUB)
                            tt(ep, ep, o, ADD)
                        elif st == 1:       # points +-1/2
                            stt(ep, d[2], 4.0, d[6])
                            stt(ep, d[4], -5.0, ep)
                            stt(o, d[3], -5.0, d[5])
                            stt(o, d[1], 4.0, o)
                            stt(em, o, -0.5, ep)
                            stt(ep, o, 0.5, ep)
                        else:               # points +-2 and 0
                            stt(ep, d[2], 0.25, d[6])
                            stt(ep, d[4], -1.25, ep)
                            stt(o, d[3], -5.0, d[1])
                            stt(o, d[5], 4.0, o)
                            stt(em, o, -0.5, ep)
                            stt(ep, o, 0.5, ep)
                            bz = bzpool.tile(
                                [P, FD], f16, tag=f"bz{ki}", name=f"bz{ki}"
                            )
                            tt(bz, d[6], d[0], SUB)
                            o2 = osc_pool.tile([P, FD], f16, tag="osc", name="osc")
                            tt(o2, d[2], d[4], SUB)
                            stt(bz, o2, 5.25, bz)
                            bx[st, ki, 2] = bz
                        bx[st, ki, 0] = bx_t[:, 0:FD]
                        bx[st, ki, 1] = bx_t[:, FD:2 * FD]

                pre_w = {}

                def prefetch_w(stage, mi):
                    u_lo, u_hi = STAGES[stage]
                    for uu in range(u_hi - u_lo):
                        w_t = wpool.tile([P, N_KI * P], f16, tag="w", name="w")
                        w_dma(
                            out=w_t, in_=w_d[mi, :, u_lo + uu, :, :]
                        )
                        pre_w[stage, mi, uu] = w_t

                def emit_migroup(stage, mi):
                    u_lo, u_hi = STAGES[stage]
                    n_u = u_hi - u_lo
                    if True:
                        psums = []
                        for uu in range(n_u):
                            w_t = pre_w.pop((stage, mi, uu), None)
                            if w_t is None:
                                w_t = wpool.tile([P, N_KI * P], f16, tag="w", name="w")
                                if schedule.startswith("v2") and stage == 0:
                                    # two ki-half chunks: the chain's ki=0
                                    # matmul only waits on the first half
                                    # (the first post-window w load sits on
                                    # the PE critical path)
                                    h = N_KI // 2 * P
                                    w_dma(
                                        out=w_t[:, 0:h],
                                        in_=w_d[mi, :, u_lo + uu, 0:N_KI // 2, :],
                                    )
                                    w_dma(
                                        out=w_t[:, h:2 * h],
                                        in_=w_d[mi, :, u_lo + uu, N_KI // 2:, :],
                                    )
                                else:
                                    w_dma(
                                        out=w_t, in_=w_d[mi, :, u_lo + uu, :, :]
                                    )
                            ps = next_bank()
                            psums.append(ps)
                            for ki in range(N_KI):
                                lhsT = w_t[:, ki * P:(ki + 1) * P]
                                nc.tensor.matmul(
                                    ps, lhsT, bx[stage, ki, uu],
                                    start=(ki == 0), stop=(ki == N_KI - 1),
                                )
                        y = y_t[mi]
                        yj = [y[:, j * FD:(j + 1) * FD] for j in range(M_TILE)]
                        if schedule.startswith("v2"):
                            # one Act eviction per migroup: pa -> SBUF; the
                            # DVE combine reads pb/pc straight from PSUM
                            # (walrus allows ONE PSUM operand per DVE op).
                            # Stage 0 skips the y2/y3 copies: stage 1 reads
                            # y0/y1 (pre-update) to seed them instead.
                            m_t = scpool.tile([P, FD], f16, tag="m0")
                            nc.scalar.copy(m_t, psums[0])
                            if stage == 0 and mi < 4:
                                # window-wave migroups: evict pb via Act too
                                # (Act is idle here; DVE reading PSUM would
                                # delay the first post-window bank release)
                                pb = sd_pool.tile([P, FD], f16, tag="mb")
                                nc.scalar.copy(pb, psums[1])
                            else:
                                pb = psums[1]
                            if stage == 0:
                                tt(yj[0], m_t, pb, ADD)
                                tt(yj[1], m_t, pb, SUB)
                            elif stage == 1:
                                s2 = sd_pool.tile([P, FD], f16, tag="s2")
                                d2 = sd_pool.tile([P, FD], f16, tag="d2")
                                tt(s2, m_t, pb, ADD)
                                tt(d2, m_t, pb, SUB)
                                stt(yj[2], s2, 0.25, yj[0])
                                tt(yj[0], s2, yj[0], ADD)
                                stt(yj[3], d2, 0.125, yj[1])
                                stt(yj[1], d2, 0.5, yj[1])
                            else:
                                sh = sd_pool.tile([P, FD], f16, tag="s2")
                                dh = sd_pool.tile([P, FD], f16, tag="d2")
                                tt(sh, m_t, pb, ADD)
                                tt(dh, m_t, pb, SUB)
                                stt(yj[2], sh, 4.0, yj[2])
                                stt(yj[1], dh, 2.0, yj[1])
                                stt(yj[3], dh, 8.0, yj[3])
                                # y0 last: it alone waits on the u6 chain
                                # (psums[2]); y1..y3 close before the last
                                # matmul so their silus overlap it
                                tt(yj[0], sh, yj[0], ADD)
                                tt(yj[0], yj[0], psums[2], ADD)
                        else:
                            # Act evicts each closed PSUM group to fp16 SBUF
                            # (walrus: only one PSUM operand per DVE op)
                            mts = []
                            for uu in range(n_u):
                                m_t = scpool.tile([P, FD], f16, tag=f"m{uu}")
                                nc.scalar.copy(m_t, psums[uu])
                                mts.append(m_t)
                            pa, pb = mts[0], mts[1]
                            if stage == 0:
                                tt(yj[0], pa, pb, ADD)
                                tt(yj[1], pa, pb, SUB)
                                nc.vector.tensor_copy(yj[2], yj[0])
                                nc.vector.tensor_copy(yj[3], yj[1])
                            elif stage == 1:
                                s2 = sd_pool.tile([P, FD], f16, tag="s2")
                                d2 = sd_pool.tile([P, FD], f16, tag="d2")
                                tt(s2, pa, pb, ADD)
                                tt(d2, pa, pb, SUB)
                                tt(yj[0], s2, yj[0], ADD)
                                stt(yj[2], s2, 0.25, yj[2])
                                stt(yj[1], d2, 0.5, yj[1])
                                stt(yj[3], d2, 0.125, yj[3])
                            else:
                                sh = sd_pool.tile([P, FD], f16, tag="s2")
                                dh = sd_pool.tile([P, FD], f16, tag="d2")
                                tt(sh, pa, pb, ADD)
                                tt(dh, pa, pb, SUB)
                                tt(yj[0], sh, yj[0], ADD)
                                tt(yj[0], mts[2], yj[0], ADD)
                                stt(yj[2], sh, 4.0, yj[2])
                                stt(yj[1], dh, 2.0, yj[1])
                                stt(yj[3], dh, 8.0, yj[3])
                        if stage == 2:
                            # finished: silu + bias, de-interleave, store
                            if schedule.startswith("v2"):
                                # two Act ops per bi: j=1..3 (ready before
                                # the u6 chain closes) then j=0 (waits on
                                # y0's psum add); fp16 out tiles; out DMAs
                                # alternate SP/Pool so the tail pipelines
                                for bi in range(B):
                                    o_t = opool.tile(
                                        [P, S_CHUNK], f16, tag="o"
                                    )
                                    # j=1..3 in one 3D-AP op: out dims
                                    # [p][j:3,s=1][t:NT,s=4], in dims
                                    # [p][j:3,s=FD][t:NT,s=1]
                                    o123 = bass.AP(
                                        o_t.tensor, o_t.offset + 1,
                                        [list(o_t.ap[0]), [1, 3],
                                         [M_TILE, NT]],
                                    )
                                    y123 = bass.AP(
                                        y.tensor,
                                        y.offset + FD + bi * NT,
                                        [list(y.ap[0]), [FD, 3], [1, NT]],
                                    )
                                    nc.scalar.activation(
                                        o123, y123,
                                        silu_fn,
                                        bias=bias_t[:, mi:mi + 1],
                                    )
                                    nc.scalar.activation(
                                        o_t[:, 0:S_CHUNK:M_TILE],
                                        y[:, bi * NT:(bi + 1) * NT],
                                        silu_fn,
                                        bias=bias_t[:, mi:mi + 1],
                                    )
                                    odma = (nc.sync.dma_start if bi % 2 == 0
                                            else nc.gpsimd.dma_start)
                                    odma(
                                        out=out_d[bi,
                                                  mi * P:(mi + 1) * P, :],
                                        in_=o_t,
                                    )
                            else:
                                for bi in range(B):
                                    o_t = opool.tile(
                                        [P, S_CHUNK], f32, tag="o"
                                    )
                                    for j in range(M_TILE):
                                        nc.scalar.activation(
                                            o_t[:, j:S_CHUNK:M_TILE],
                                            y[:, j * FD + bi * NT:
                                               j * FD + (bi + 1) * NT],
                                            silu_fn,
                                            bias=bias_t[:, mi:mi + 1],
                                        )
                                    nc.sync.dma_start(
                                        out=out_d[bi,
                                                  mi * P:(mi + 1) * P, :],
                                        in_=o_t,
                                    )

                if schedule == "fused":
                    # one plane load feeds stage0+stage1 transforms up
                    # front; stage2 transforms interleave into stage1's
                    # matmul loop (PE never waits at stage boundaries,
                    # plane DMA read twice per pass).
                    for ki in range(N_KI):
                        emit_transform([0, 1], ki)
                    for mi in range(N_MI):
                        emit_migroup(0, mi)
                    for mi in range(N_MI):
                        emit_migroup(1, mi)
                        emit_transform([2], mi)
                    for mi in range(N_MI):
                        emit_migroup(2, mi)
                else:
                    # spread: only stage0 transforms up front (shortest
                    # PE fill); stage s+1 transforms interleave into
                    # stage s's matmul loop (plane DMA read 3x per pass).
                    if schedule in ("spread2", "spread3", "spread5"):
                        # first two mi's weights ahead of the plane DMAs
                        # on SP's FIFO so PE can start with the first BX
                        prefetch_w(0, 0)
                        prefetch_w(0, 1)
                    elif schedule.startswith("v2"):
                        # Pool queue runs parallel to SP: prefetch the
                        # first four migroups' weights (fills the 8-buf
                        # ring: 8 chains ride the plane-arrival window)
                        for pmi in range(4):
                            prefetch_w(0, pmi)
                        w_dma(out=bias_t, in_=bias_d)
                        # warm up Act off the critical path: its first op
                        # costs ~2us (pipe ramp) and otherwise lands on the
                        # first post-window PSUM eviction
                        warm_t = bpool.tile([P, N_MI], f32, tag="warm")
                        nc.scalar.copy(warm_t, bias_t)
                    for ki in range(N_KI):
                        emit_transform([0], ki)
                    for mi in range(N_MI):
                        emit_migroup(0, mi)
                        emit_transform([1], mi)
                    for mi in range(N_MI):
                        emit_migroup(1, mi)
                        emit_transform([2], mi)
                    for mi in range(N_MI):
                        emit_migroup(2, mi)
    nc.compile()
    return nc


def prep_inputs(x, weight, bias):
    """Full fp32 inputs -> per-core in_maps with winograd host transforms."""
    x = np.asarray(x, dtype=np.float32)
    weight = np.asarray(weight, dtype=np.float32)
    bias = np.asarray(bias, dtype=np.float32)

    G = winograd_G()
    # GW[u, co, ci] fp16, laid out [mi, p(ci), u, ki, f(co)]
    GW = np.einsum('ut,oit->uoi', G, weight).astype(np.float16)
    GW = GW.reshape(U, N_MI, P, N_KI, P)          # (u, mi, f, ki, p)
    w_host = np.ascontiguousarray(GW.transpose(1, 4, 0, 3, 2))

    bias2 = np.ascontiguousarray(bias.reshape(N_MI, P).T)  # (P, n_mi)

    xp = np.pad(x, ((0, 0), (0, 0), (HALO, 0))).astype(np.float16)  # (B,CI,S+3)
    in_maps = []
    for c in range(N_CORES):
        xc = xp[:, :, c * S_CHUNK: c * S_CHUNK + S_CHUNK + HALO]  # (B,CI,515)
        # planes[v][t] = xc[..., 4t+v], t<128 -> layout [ki,p,v,bi,t]
        pl = np.empty((B, C_IN, U, NT), dtype=np.float16)
        for v in range(U):
            pl[:, :, v, :] = xc[:, :, v: v + 4 * NT: 4][:, :, :NT]
        pl = pl.reshape(B, N_KI, P, U, NT)
        pl = np.ascontiguousarray(pl.transpose(1, 2, 3, 0, 4))  # ki,p,v,bi,t
        in_maps.append({"x": pl, "w": w_host, "bias": bias2})
    return in_maps


def build_for_bench(x, weight, bias, reps=1, schedule="v2"):
    nc = build_winograd_nc(reps=reps, schedule=schedule)
    in_maps = prep_inputs(x, weight, bias)
    return nc, in_maps


def kernel(x, weight, bias):
    nc, in_maps = build_for_bench(x, weight, bias, reps=1)
    global LAST_RESULT
    res = run_bass_kernel_spmd(
        nc, in_maps, core_ids=list(range(N_CORES)), trace=PROFILE
    )
    LAST_RESULT = res
    out = np.concatenate([r["out"] for r in res.results], axis=2)
    return np.ascontiguousarray(out.astype(np.float32, copy=False))


PROFILE = False
LAST_RESULT = None

